# revision 11
# baseline (speedup 1.0000x reference)
"""Trainium2 Bass kernel for CFKANLayer (Chebyshev KAN layer).

Computes y[n,o] = sum_{d,k} T_k(tanh(x[n,d])) * C[o,d,k] + bias[o]
with N=65536, D=256, O=256, K=8, data-parallel over 8 NeuronCores.

Math: with t = tanh(x) and u = T_2(t) = 2t^2-1, every T_k (k=1..7) is a
small-integer combination of the 7 streams {t, u, tu, u^2, tu^2, u^3, tu^3}
(via T_{2m}=T_m(u) and degree reduction), so the whole layer folds into a
single 7*256-contraction matmul with host-folded (exact, f64) weights:
    T1 = t                    T5 = 4tu^2 - 2tu - t
    T2 = u                    T6 = 4u^3 - 3u
    T3 = 2tu - t              T7 = 8tu^3 - 4tu^2 - 4tu + t
    T4 = 2u^2 - 1 (bias)      T0 = 1 (bias)

Per-core layout (8192 tokens): the host supplies x^T (d-major, fp16), so
DMA loads land directly in (d, n) layout -- no on-device transposes.
Per 512-token block:
  DMA x^T tile -> ACT: tanh, s=t^2, u2=u^2; DVE: u=2s-1, tu, u3, tu2, tu3
  -> PE: per o-half, 14 accumulating fp16 matmuls (128d x 128o weight
  stationary, 128d x 512n stream moving) -> psum y^T (128o, 512n) ->
  ACT/DVE evac with exact f32 per-partition bias add, fp16 out ->
  DMA out to y^T (fp16) in DRAM.  Host transposes/upcasts the result.
The stream chain for block b+2 is emitted before block b's matmuls so
ACT/DVE FIFO work is never stuck behind a PE-gated psum evacuation.
"""

import os
import sys

import numpy as np

sys.path.insert(0, "/opt/trn_rl_repo")

N_FULL, D, O, K = 65536, 256, 256, 8
NCORES = 8
BLK = 512          # tokens per pipeline block
NSTREAMS = 7
NCH = NSTREAMS * 2 # weight chunks: (stream, d_chunk of 128)

# stash of the last BassKernelResults (test.py reads exec_time_ns)
LAST_RESULTS = None

_PROGRAM_CACHE = {}


def _fold_weights(cheby_coeffs, bias):
    """Host-side exact (f64) weight folding into the u-basis. Returns
    (W14, bt): W14[(s,dc), dd, o] fp16 weight chunks for streams
    [t, u, tu, u2, tu2, u3, tu3], bt the (2, 128) f32 effective bias."""
    C = cheby_coeffs.astype(np.float64)              # (O, D, K)
    C0, C1, C2, C3, C4, C5, C6, C7 = [C[:, :, k] for k in range(8)]
    w_t = C1 - C3 - C5 + C7
    w_u = C2 - 3 * C6
    w_tu = 2 * C3 - 2 * C5 - 4 * C7
    w_u2 = 2 * C4
    w_tu2 = 4 * C5 - 4 * C7
    w_u3 = 4 * C6
    w_tu3 = 8 * C7
    W = np.stack([w_t, w_u, w_tu, w_u2, w_tu2, w_u3, w_tu3], axis=0)  # (7, O, D)
    # chunk layout: (s, dc) -> (128 dd, O) with d = dc*128 + dd
    Wc = W.reshape(NSTREAMS, O, 2, 128).transpose(0, 2, 3, 1).reshape(NCH, 128, O)
    bias_eff = bias.astype(np.float64).reshape(-1)[:O] + (C0 - C4).sum(axis=1)
    bt = bias_eff.reshape(2, 128).astype(np.float32)
    return Wc.astype(np.float16), bt


def build_program(nshard, debug=False, reps=1, blk=None, lookahead=2):
    """Build the per-core Bass/Tile program for an `nshard`-token shard.

    reps>1 wraps the whole pipeline in a dynamic loop (identical work each
    iteration) -- used only by the timing harness to isolate device time
    from RPC/transfer overhead via differential measurement."""
    import concourse.bacc as bacc
    import concourse.mybir as mybir
    import concourse.tile as tile
    from contextlib import ExitStack

    # dev-only ablation switches for the timing harness
    skip_dma = os.environ.get("KERNEL_SKIP_DMA", "0") == "1"
    skip_mm = os.environ.get("KERNEL_SKIP_MM", "0") == "1"
    skip_streams = os.environ.get("KERNEL_SKIP_STREAMS", "0") == "1"

    FP16 = mybir.dt.float16
    F32 = mybir.dt.float32
    AF = mybir.ActivationFunctionType
    ALU = mybir.AluOpType

    BLK = blk if blk is not None else globals()["BLK"]
    assert nshard % BLK == 0
    nblk = nshard // BLK

    nc = bacc.Bacc("TRN2", target_bir_lowering=False, debug=debug)
    xt_d = nc.dram_tensor("xt", [D, nshard], FP16, kind="ExternalInput")
    w = nc.dram_tensor("w", [NCH, 128, O], FP16, kind="ExternalInput")
    bt = nc.dram_tensor("bt", [2, 128], F32, kind="ExternalInput")
    yt = nc.dram_tensor("yt", [O, nshard], FP16, kind="ExternalOutput")

    with tile.TileContext(nc) as tc, ExitStack() as ctx:
        constp = ctx.enter_context(tc.tile_pool(name="const", bufs=1))
        wpool = ctx.enter_context(tc.tile_pool(name="wpool", bufs=1))
        xin = ctx.enter_context(tc.tile_pool(name="xin", bufs=4))
        sp = ctx.enter_context(tc.tile_pool(name="stream", bufs=3))
        yp = ctx.enter_context(tc.tile_pool(name="yout", bufs=4))
        pyp = ctx.enter_context(tc.tile_pool(name="py", bufs=4, space="PSUM"))

        bias_og = []
        for og in range(2):
            btile = constp.tile([128, 1], F32, tag=f"bias{og}", name=f"bias{og}")
            nc.sync.dma_start(out=btile, in_=bt[og].unsqueeze(1))
            bias_og.append(btile)
        # all 14 weight chunks in one 896KB DMA (row-major per partition),
        # issued on the ACT HWDGE ring so it doesn't serialize ahead of the
        # first x-block loads on the SP ring
        wt_all = wpool.tile([128, NCH, O], FP16, tag="w", name="w")
        nc.scalar.dma_start(out=wt_all, in_=w[:, :].rearrange("c p o -> p c o"))
        wt = [wt_all[:, c, :] for c in range(NCH)]

        # HAM warmup: dummy matmuls overlapping the DMA prologue so the PE
        # clock gate is already at 8/8 when the first real matmul issues.
        # Runs once (outside the reps loop); reads a gpsimd-memset tile,
        # writes a psum tile that is never read.
        warm = ctx.enter_context(tc.tile_pool(name="warm", bufs=1, space="PSUM"))
        wsrc = constp.tile([128, 512], FP16, tag="warmsrc")
        nc.gpsimd.memset(wsrc, 0.5)
        pwarm = warm.tile([128, 512], F32, tag="pwarm")
        for i in range(10):
            nc.tensor.matmul(pwarm, wsrc[:, 0:128], wsrc,
                             start=(i == 0), stop=(i == 9))

        # x^T rows: d = j*128 + p, free dims (j, n)
        xv = xt_d[:, :].rearrange("(j p) n -> p j n", p=128)

        x_const = None
        if skip_dma:
            x_const = constp.tile([128, 2 * BLK], FP16, tag="xconst")
            nc.gpsimd.memset(x_const, 0.25)

        str_const = None
        if skip_streams:
            str_const = constp.tile([128, 2 * BLK], FP16, tag="strconst")
            nc.gpsimd.memset(str_const, 0.125)

        def prepare_block(bI):
            """DMA + tanh + all stream computation for block bI, in
            (d, n) layout throughout.  free layout = j*BLK + n."""
            if skip_streams:
                return [str_const] * NSTREAMS
            if skip_dma:
                x_in = x_const
            else:
                x_in = xin.tile([128, 2, BLK], FP16, tag="x")
                nc.sync.dma_start(
                    out=x_in, in_=xv[:, :, bI * BLK:(bI + 1) * BLK]
                )
                x_in = x_in[:, :, :].rearrange("p j n -> p (j n)")
            t = sp.tile([128, 2 * BLK], FP16, tag="t")
            nc.scalar.activation(out=t, in_=x_in, func=AF.Tanh)
            s = sp.tile([128, 2 * BLK], FP16, tag="s")
            nc.scalar.activation(out=s, in_=t, func=AF.Square)
            u = sp.tile([128, 2 * BLK], FP16, tag="u")
            nc.vector.tensor_scalar(out=u, in0=s, scalar1=2.0, scalar2=-1.0,
                                    op0=ALU.mult, op1=ALU.add)
            u2 = sp.tile([128, 2 * BLK], FP16, tag="u2")
            nc.scalar.activation(out=u2, in_=u, func=AF.Square)
            tu = sp.tile([128, 2 * BLK], FP16, tag="tu")
            nc.vector.tensor_tensor(out=tu, in0=t, in1=u, op=ALU.mult)
            u3 = sp.tile([128, 2 * BLK], FP16, tag="u3")
            nc.vector.tensor_tensor(out=u3, in0=u, in1=u2, op=ALU.mult)
            tu2 = sp.tile([128, 2 * BLK], FP16, tag="tu2")
            nc.vector.tensor_tensor(out=tu2, in0=t, in1=u2, op=ALU.mult)
            tu3 = sp.tile([128, 2 * BLK], FP16, tag="tu3")
            nc.vector.tensor_tensor(out=tu3, in0=tu, in1=u2, op=ALU.mult)
            return [t, u, tu, u2, tu2, u3, tu3]

        NH = BLK // 512  # psum-bank-sized n-slices per block

        def mm_block(bI, streams):
            if skip_mm:
                return
            for og in range(2):
                pw = pyp.tile([128, BLK], F32, tag="pw")
                for nh in range(NH):
                    kk = 0
                    for si in range(NSTREAMS):
                        for dc in range(2):
                            nc.tensor.matmul(
                                pw[:, nh * 512:(nh + 1) * 512],
                                wt[si * 2 + dc][:, og * 128:(og + 1) * 128],
                                streams[si][:, dc * BLK + nh * 512:
                                            dc * BLK + (nh + 1) * 512],
                                start=(kk == 0), stop=(kk == 2 * NSTREAMS - 1),
                            )
                            kk += 1
                yo = yp.tile([128, BLK], FP16, tag=f"yo{og}", name=f"yo{og}")
                # evac with per-partition f32 bias add; og0 on ACT, og1 on
                # DVE so neither FIFO stacks two PE-gated ops per block
                if og == 0:
                    nc.scalar.activation(out=yo, in_=pw, func=AF.Identity,
                                         bias=bias_og[og], scale=1.0)
                else:
                    nc.vector.tensor_scalar(out=yo, in0=pw,
                                            scalar1=bias_og[og], scalar2=None,
                                            op0=ALU.add)
                if not skip_dma:
                    nc.sync.dma_start(
                        out=yt[og * 128:(og + 1) * 128, bI * BLK:(bI + 1) * BLK],
                        in_=yo,
                    )

        def run_pipeline():
            # lookahead blocks: stream chain latency (~6.5us incl DMA) is
            # about one MM-block (6us), so one block ahead is not enough.
            pending = [prepare_block(b) for b in range(min(lookahead, nblk))]
            for bI in range(nblk):
                if bI + lookahead < nblk:
                    pending.append(prepare_block(bI + lookahead))
                mm_block(bI, pending.pop(0))

        if reps > 1:
            with tc.For_i(0, reps, 1):
                run_pipeline()
        else:
            run_pipeline()

    nc.compile()
    return nc


def kernel(x, cheby_coeffs, bias):
    global LAST_RESULTS
    # NTFF trace hooks (antenv.axon_hooks) are absent in this container;
    # make sure nothing flips tracing on under us.
    os.environ["BASS_NEVER_TRACE"] = "1"
    from concourse.bass_utils import run_bass_kernel_spmd

    x = np.asarray(x, dtype=np.float32)
    n_tok = x.shape[0]
    assert n_tok % NCORES == 0
    nshard = n_tok // NCORES

    W14, bt = _fold_weights(np.asarray(cheby_coeffs), np.asarray(bias))
    x16 = x.astype(np.float16)

    key = nshard
    if key not in _PROGRAM_CACHE:
        _PROGRAM_CACHE[key] = build_program(nshard)
    nc = _PROGRAM_CACHE[key]

    in_maps = [
        {"xt": np.ascontiguousarray(x16[c * nshard:(c + 1) * nshard].T),
         "w": W14, "bt": bt}
        for c in range(NCORES)
    ]
    res = run_bass_kernel_spmd(nc, in_maps, list(range(NCORES)))
    LAST_RESULTS = res
    y = np.concatenate(
        [res.results[c]["yt"].T.astype(np.float32) for c in range(NCORES)],
        axis=0,
    )
    return y


# revision 12
# speedup vs baseline: 2.9380x; 2.9380x over previous
"""Trainium2 Bass kernel for CFKANLayer (Chebyshev KAN layer).

Computes y[n,o] = sum_{d,k} T_k(tanh(x[n,d])) * C[o,d,k] + bias[o]
with N=65536, D=256, O=256, K=8, data-parallel over 8 NeuronCores.

Math: with t = tanh(x) and u = T_2(t) = 2t^2-1, every T_k (k=1..7) is a
small-integer combination of the 7 streams {t, u, tu, u^2, tu^2, u^3, tu^3}
(via T_{2m}=T_m(u) and degree reduction), so the whole layer folds into a
single 7*256-contraction matmul with host-folded (exact, f64) weights:
    T1 = t                    T5 = 4tu^2 - 2tu - t
    T2 = u                    T6 = 4u^3 - 3u
    T3 = 2tu - t              T7 = 8tu^3 - 4tu^2 - 4tu + t
    T4 = 2u^2 - 1 (bias)      T0 = 1 (bias)

Per-core layout (8192 tokens): the host supplies x^T (d-major, fp16), so
DMA loads land directly in (d, n) layout -- no on-device transposes.
Per 512-token block:
  DMA x^T tile -> ACT: tanh, s=t^2, u2=u^2; DVE: u=2s-1, tu, u3, tu2, tu3
  -> PE: per o-half, 14 accumulating fp16 matmuls (128d x 128o weight
  stationary, 128d x 512n stream moving) -> psum y^T (128o, 512n) ->
  ACT/DVE evac with exact f32 per-partition bias add, fp16 out ->
  DMA out to y^T (fp16) in DRAM.  Host transposes/upcasts the result.
The stream chain for block b+2 is emitted before block b's matmuls so
ACT/DVE FIFO work is never stuck behind a PE-gated psum evacuation.
"""

import os
import sys

import numpy as np

sys.path.insert(0, "/opt/trn_rl_repo")

N_FULL, D, O, K = 65536, 256, 256, 8
NCORES = 8
BLK = 512          # tokens per pipeline block
NSTREAMS = 7
NCH = NSTREAMS * 2 # weight chunks: (stream, d_chunk of 128)

# stash of the last BassKernelResults (test.py reads exec_time_ns)
LAST_RESULTS = None

_PROGRAM_CACHE = {}


def _fold_weights(cheby_coeffs, bias):
    """Host-side exact (f64) weight folding into the u-basis. Returns
    (W14, bt): W14[(s,dc), dd, o] fp16 weight chunks for streams
    [t, u, tu, u2, tu2, u3, tu3], bt the (2, 128) f32 effective bias."""
    C = cheby_coeffs.astype(np.float64)              # (O, D, K)
    C0, C1, C2, C3, C4, C5, C6, C7 = [C[:, :, k] for k in range(8)]
    w_t = C1 - C3 - C5 + C7
    w_u = C2 - 3 * C6
    w_tu = 2 * C3 - 2 * C5 - 4 * C7
    w_u2 = 2 * C4
    w_tu2 = 4 * C5 - 4 * C7
    w_u3 = 4 * C6
    w_tu3 = 8 * C7
    W = np.stack([w_t, w_u, w_tu, w_u2, w_tu2, w_u3, w_tu3], axis=0)  # (7, O, D)
    # chunk layout: (s, dc) -> (128 dd, O) with d = dc*128 + dd
    Wc = W.reshape(NSTREAMS, O, 2, 128).transpose(0, 2, 3, 1).reshape(NCH, 128, O)
    bias_eff = bias.astype(np.float64).reshape(-1)[:O] + (C0 - C4).sum(axis=1)
    bt = bias_eff.reshape(2, 128).astype(np.float32)
    return Wc.astype(np.float16), bt


def build_program(nshard, debug=False, reps=1, blk=None, lookahead=2):
    """Build the per-core Bass/Tile program for an `nshard`-token shard.

    reps>1 wraps the whole pipeline in a dynamic loop (identical work each
    iteration) -- used only by the timing harness to isolate device time
    from RPC/transfer overhead via differential measurement."""
    import concourse.bacc as bacc
    import concourse.mybir as mybir
    import concourse.tile as tile
    from contextlib import ExitStack

    # dev-only ablation switches for the timing harness
    skip_dma = os.environ.get("KERNEL_SKIP_DMA", "0") == "1"
    skip_mm = os.environ.get("KERNEL_SKIP_MM", "0") == "1"
    skip_streams = os.environ.get("KERNEL_SKIP_STREAMS", "0") == "1"

    FP16 = mybir.dt.float16
    F32 = mybir.dt.float32
    AF = mybir.ActivationFunctionType
    ALU = mybir.AluOpType

    BLK = blk if blk is not None else globals()["BLK"]
    assert nshard % BLK == 0
    nblk = nshard // BLK

    nc = bacc.Bacc("TRN2", target_bir_lowering=False, debug=debug)
    xt_d = nc.dram_tensor("xt", [D, nshard], FP16, kind="ExternalInput")
    w = nc.dram_tensor("w", [NCH, 128, O], FP16, kind="ExternalInput")
    bt = nc.dram_tensor("bt", [2, 128], F32, kind="ExternalInput")
    yt = nc.dram_tensor("yt", [O, nshard], FP16, kind="ExternalOutput")

    with tile.TileContext(nc) as tc, ExitStack() as ctx:
        constp = ctx.enter_context(tc.tile_pool(name="const", bufs=1))
        wpool = ctx.enter_context(tc.tile_pool(name="wpool", bufs=1))
        xin = ctx.enter_context(tc.tile_pool(name="xin", bufs=4))
        sp = ctx.enter_context(tc.tile_pool(name="stream", bufs=3))
        yp = ctx.enter_context(tc.tile_pool(name="yout", bufs=4))
        pyp = ctx.enter_context(tc.tile_pool(name="py", bufs=4, space="PSUM"))

        bias_og = []
        for og in range(2):
            btile = constp.tile([128, 1], F32, tag=f"bias{og}", name=f"bias{og}")
            nc.sync.dma_start(out=btile, in_=bt[og].unsqueeze(1))
            bias_og.append(btile)
        # all 14 weight chunks in one 896KB DMA (row-major per partition),
        # issued on the ACT HWDGE ring so it doesn't serialize ahead of the
        # first x-block loads on the SP ring
        wt_all = wpool.tile([128, NCH, O], FP16, tag="w", name="w")
        nc.scalar.dma_start(out=wt_all, in_=w[:, :].rearrange("c p o -> p c o"))
        wt = [wt_all[:, c, :] for c in range(NCH)]

        # HAM warmup: dummy matmuls overlapping the DMA prologue so the PE
        # clock gate is already at 8/8 when the first real matmul issues.
        # Runs once (outside the reps loop); reads a gpsimd-memset tile,
        # writes a psum tile that is never read.
        warm = ctx.enter_context(tc.tile_pool(name="warm", bufs=1, space="PSUM"))
        wsrc = constp.tile([128, 512], FP16, tag="warmsrc")
        nc.gpsimd.memset(wsrc, 0.5)
        pwarm = warm.tile([128, 512], F32, tag="pwarm")
        for i in range(10):
            nc.tensor.matmul(pwarm, wsrc[:, 0:128], wsrc,
                             start=(i == 0), stop=(i == 9))

        # x^T rows: d = j*128 + p, free dims (j, n)
        xv = xt_d[:, :].rearrange("(j p) n -> p j n", p=128)

        x_const = None
        if skip_dma:
            x_const = constp.tile([128, 2 * BLK], FP16, tag="xconst")
            nc.gpsimd.memset(x_const, 0.25)

        str_const = None
        if skip_streams:
            str_const = constp.tile([128, 2 * BLK], FP16, tag="strconst")
            nc.gpsimd.memset(str_const, 0.125)

        def prepare_block(st, ntok):
            """DMA + tanh + all stream computation for tokens [st, st+ntok),
            in (d, n) layout throughout.  Tiles are allocated at the max
            block size; only the first 2*ntok columns are used, so the flat
            free layout is j*ntok + n."""
            if skip_streams:
                return [str_const] * NSTREAMS
            if skip_dma:
                x_in = x_const[:, 0:2 * ntok]
            else:
                xt_t = xin.tile([128, 2 * BLK], FP16, tag="x")
                nc.sync.dma_start(
                    out=xt_t[:, 0:2 * ntok].rearrange("p (j n) -> p j n", j=2),
                    in_=xv[:, :, st:st + ntok],
                )
                x_in = xt_t[:, 0:2 * ntok]
            w2 = 2 * ntok
            t = sp.tile([128, 2 * BLK], FP16, tag="t")
            nc.scalar.activation(out=t[:, 0:w2], in_=x_in, func=AF.Tanh)
            s = sp.tile([128, 2 * BLK], FP16, tag="s")
            nc.scalar.activation(out=s[:, 0:w2], in_=t[:, 0:w2], func=AF.Square)
            u = sp.tile([128, 2 * BLK], FP16, tag="u")
            nc.vector.tensor_scalar(out=u[:, 0:w2], in0=s[:, 0:w2],
                                    scalar1=2.0, scalar2=-1.0,
                                    op0=ALU.mult, op1=ALU.add)
            u2 = sp.tile([128, 2 * BLK], FP16, tag="u2")
            nc.scalar.activation(out=u2[:, 0:w2], in_=u[:, 0:w2], func=AF.Square)
            tu = sp.tile([128, 2 * BLK], FP16, tag="tu")
            nc.vector.tensor_tensor(out=tu[:, 0:w2], in0=t[:, 0:w2],
                                    in1=u[:, 0:w2], op=ALU.mult)
            u3 = sp.tile([128, 2 * BLK], FP16, tag="u3")
            nc.vector.tensor_tensor(out=u3[:, 0:w2], in0=u[:, 0:w2],
                                    in1=u2[:, 0:w2], op=ALU.mult)
            tu2 = sp.tile([128, 2 * BLK], FP16, tag="tu2")
            nc.vector.tensor_tensor(out=tu2[:, 0:w2], in0=t[:, 0:w2],
                                    in1=u2[:, 0:w2], op=ALU.mult)
            tu3 = sp.tile([128, 2 * BLK], FP16, tag="tu3")
            nc.vector.tensor_tensor(out=tu3[:, 0:w2], in0=tu[:, 0:w2],
                                    in1=u2[:, 0:w2], op=ALU.mult)
            return [t, u, tu, u2, tu2, u3, tu3]

        def mm_block(st, ntok, streams):
            if skip_mm:
                return
            nh_tot = (ntok + 511) // 512
            for og in range(2):
                pw = pyp.tile([128, BLK], F32, tag="pw")
                for nh in range(nh_tot):
                    n0, n1 = nh * 512, min((nh + 1) * 512, ntok)
                    kk = 0
                    for si in range(NSTREAMS):
                        for dc in range(2):
                            nc.tensor.matmul(
                                pw[:, n0:n1],
                                wt[si * 2 + dc][:, og * 128:(og + 1) * 128],
                                streams[si][:, dc * ntok + n0:dc * ntok + n1],
                                start=(kk == 0), stop=(kk == 2 * NSTREAMS - 1),
                            )
                            kk += 1
                yo = yp.tile([128, BLK], FP16, tag=f"yo{og}", name=f"yo{og}")
                # evac with per-partition f32 bias add; og0 on ACT, og1 on
                # DVE so neither FIFO stacks two PE-gated ops per block
                if og == 0:
                    nc.scalar.activation(out=yo[:, 0:ntok], in_=pw[:, 0:ntok],
                                         func=AF.Identity,
                                         bias=bias_og[og], scale=1.0)
                else:
                    nc.vector.tensor_scalar(out=yo[:, 0:ntok], in0=pw[:, 0:ntok],
                                            scalar1=bias_og[og], scalar2=None,
                                            op0=ALU.add)
                if not skip_dma:
                    nc.sync.dma_start(
                        out=yt[og * 128:(og + 1) * 128, st:st + ntok],
                        in_=yo[:, 0:ntok],
                    )

        # Tapered block list: small first/last blocks shorten pipeline fill
        # (first streams ready sooner) and drain (last evac+DMA smaller).
        if nblk >= 4:
            blocks = ([(0, 256), (256, 256)]
                      + [(512 + i * BLK, BLK) for i in range(nblk - 2)]
                      + [(nshard - 512, 256), (nshard - 256, 256)])
        else:
            blocks = [(i * BLK, BLK) for i in range(nblk)]
        assert sum(n for _, n in blocks) == nshard

        def run_pipeline():
            # lookahead blocks: stream chain latency (~6.5us incl DMA) is
            # about one MM-block (6us), so one block ahead is not enough.
            nb = len(blocks)
            pending = [prepare_block(*blocks[b]) for b in range(min(lookahead, nb))]
            for bI in range(nb):
                if bI + lookahead < nb:
                    pending.append(prepare_block(*blocks[bI + lookahead]))
                mm_block(*blocks[bI], pending.pop(0))

        if reps > 1:
            with tc.For_i(0, reps, 1):
                run_pipeline()
        else:
            run_pipeline()

    nc.compile()
    return nc


def kernel(x, cheby_coeffs, bias):
    global LAST_RESULTS
    # NTFF trace hooks (antenv.axon_hooks) are absent in this container;
    # make sure nothing flips tracing on under us.
    os.environ["BASS_NEVER_TRACE"] = "1"
    from concourse.bass_utils import run_bass_kernel_spmd

    x = np.asarray(x, dtype=np.float32)
    n_tok = x.shape[0]
    assert n_tok % NCORES == 0
    nshard = n_tok // NCORES

    W14, bt = _fold_weights(np.asarray(cheby_coeffs), np.asarray(bias))
    x16 = x.astype(np.float16)

    key = nshard
    if key not in _PROGRAM_CACHE:
        _PROGRAM_CACHE[key] = build_program(nshard)
    nc = _PROGRAM_CACHE[key]

    in_maps = [
        {"xt": np.ascontiguousarray(x16[c * nshard:(c + 1) * nshard].T),
         "w": W14, "bt": bt}
        for c in range(NCORES)
    ]
    res = run_bass_kernel_spmd(nc, in_maps, list(range(NCORES)))
    LAST_RESULTS = res
    y = np.concatenate(
        [res.results[c]["yt"].T.astype(np.float32) for c in range(NCORES)],
        axis=0,
    )
    return y


# revision 13
# speedup vs baseline: 3.4439x; 1.1722x over previous
"""Trainium2 Bass kernel for CFKANLayer (Chebyshev KAN layer).

Computes y[n,o] = sum_{d,k} T_k(tanh(x[n,d])) * C[o,d,k] + bias[o]
with N=65536, D=256, O=256, K=8, data-parallel over 8 NeuronCores.

Math: with t = tanh(x) and u = T_2(t) = 2t^2-1, every T_k (k=1..7) is a
small-integer combination of the 7 streams {t, u, tu, u^2, tu^2, u^3, tu^3}
(via T_{2m}=T_m(u) and degree reduction), so the whole layer folds into a
single 7*256-contraction matmul with host-folded (exact, f64) weights:
    T1 = t                    T5 = 4tu^2 - 2tu - t
    T2 = u                    T6 = 4u^3 - 3u
    T3 = 2tu - t              T7 = 8tu^3 - 4tu^2 - 4tu + t
    T4 = 2u^2 - 1 (bias)      T0 = 1 (bias)

Per-core layout (8192 tokens): the host supplies x^T (d-major, fp16), so
DMA loads land directly in (d, n) layout -- no on-device transposes.
Per 512-token block:
  DMA x^T tile -> ACT: tanh, s=t^2, u2=u^2; DVE: u=2s-1, tu, u3, tu2, tu3
  -> PE: per o-half, 14 accumulating fp16 matmuls (128d x 128o weight
  stationary, 128d x 512n stream moving) -> psum y^T (128o, 512n) ->
  ACT/DVE evac with exact f32 per-partition bias add, fp16 out ->
  DMA out to y^T (fp16) in DRAM.  Host transposes/upcasts the result.
The stream chain for block b+2 is emitted before block b's matmuls so
ACT/DVE FIFO work is never stuck behind a PE-gated psum evacuation.
"""

import os
import sys

import numpy as np

sys.path.insert(0, "/opt/trn_rl_repo")

N_FULL, D, O, K = 65536, 256, 256, 8
NCORES = 8
BLK = 512          # tokens per pipeline block
NSTREAMS = 7
NCH = NSTREAMS * 2 # weight chunks: (stream, d_chunk of 128)

# stash of the last BassKernelResults (debugging convenience)
LAST_RESULTS = None

_PROGRAM_CACHE = {}


def _fold_weights(cheby_coeffs, bias):
    """Host-side exact (f64) weight folding into the u-basis. Returns
    (W14, bt): W14[(s,dc), dd, o] fp16 weight chunks for streams
    [t, u, tu, u2, tu2, u3, tu3], bt the (2, 128) f32 effective bias."""
    C = cheby_coeffs.astype(np.float64)              # (O, D, K)
    C0, C1, C2, C3, C4, C5, C6, C7 = [C[:, :, k] for k in range(8)]
    w_t = C1 - C3 - C5 + C7
    w_u = C2 - 3 * C6
    w_tu = 2 * C3 - 2 * C5 - 4 * C7
    w_u2 = 2 * C4
    w_tu2 = 4 * C5 - 4 * C7
    w_u3 = 4 * C6
    w_tu3 = 8 * C7
    W = np.stack([w_t, w_u, w_tu, w_u2, w_tu2, w_u3, w_tu3], axis=0)  # (7, O, D)
    # chunk layout: (s, dc) -> (128 dd, O) with d = dc*128 + dd
    Wc = W.reshape(NSTREAMS, O, 2, 128).transpose(0, 2, 3, 1).reshape(NCH, 128, O)
    bias_eff = bias.astype(np.float64).reshape(-1)[:O] + (C0 - C4).sum(axis=1)
    bt = bias_eff.reshape(2, 128).astype(np.float32)
    return Wc.astype(np.float16), bt


def build_program(nshard, debug=False, reps=1, blk=None, lookahead=2):
    """Build the per-core Bass/Tile program for an `nshard`-token shard.

    reps>1 wraps the whole pipeline in a dynamic loop (identical work each
    iteration) -- used only by the timing harness to isolate device time
    from RPC/transfer overhead via differential measurement."""
    import concourse.bacc as bacc
    import concourse.mybir as mybir
    import concourse.tile as tile
    from contextlib import ExitStack

    # dev-only ablation switches for the timing harness
    skip_dma = os.environ.get("KERNEL_SKIP_DMA", "0") == "1"
    skip_mm = os.environ.get("KERNEL_SKIP_MM", "0") == "1"
    skip_streams = os.environ.get("KERNEL_SKIP_STREAMS", "0") == "1"

    FP16 = mybir.dt.float16
    F32 = mybir.dt.float32
    AF = mybir.ActivationFunctionType
    ALU = mybir.AluOpType

    BLK = blk if blk is not None else globals()["BLK"]
    assert nshard % BLK == 0
    nblk = nshard // BLK

    nc = bacc.Bacc("TRN2", target_bir_lowering=False, debug=debug)
    xt_d = nc.dram_tensor("xt", [D, nshard], FP16, kind="ExternalInput")
    w = nc.dram_tensor("w", [NCH, 128, O], FP16, kind="ExternalInput")
    bt = nc.dram_tensor("bt", [2, 128], F32, kind="ExternalInput")
    yt = nc.dram_tensor("yt", [O, nshard], FP16, kind="ExternalOutput")

    with tile.TileContext(nc) as tc, ExitStack() as ctx:
        constp = ctx.enter_context(tc.tile_pool(name="const", bufs=1))
        wpool = ctx.enter_context(tc.tile_pool(name="wpool", bufs=1))
        xin = ctx.enter_context(tc.tile_pool(name="xin", bufs=4))
        sp = ctx.enter_context(tc.tile_pool(name="stream", bufs=3))
        yp = ctx.enter_context(tc.tile_pool(name="yout", bufs=4))
        pyp = ctx.enter_context(tc.tile_pool(name="py", bufs=4, space="PSUM"))

        bias_og = []
        for og in range(2):
            btile = constp.tile([128, 1], F32, tag=f"bias{og}", name=f"bias{og}")
            nc.sync.dma_start(out=btile, in_=bt[og].unsqueeze(1))
            bias_og.append(btile)
        # all 14 weight chunks in one 896KB DMA (row-major per partition),
        # issued on the ACT HWDGE ring so it doesn't serialize ahead of the
        # first x-block loads on the SP ring
        wt_all = wpool.tile([128, NCH, O], FP16, tag="w", name="w")
        nc.scalar.dma_start(out=wt_all, in_=w[:, :].rearrange("c p o -> p c o"))
        wt = [wt_all[:, c, :] for c in range(NCH)]

        # HAM warmup: dummy matmuls overlapping the DMA prologue so the PE
        # clock gate is already at 8/8 when the first real matmul issues.
        # Runs once (outside the reps loop); reads a gpsimd-memset tile,
        # writes a psum tile that is never read.
        warm = ctx.enter_context(tc.tile_pool(name="warm", bufs=1, space="PSUM"))
        wsrc = constp.tile([128, 512], FP16, tag="warmsrc")
        nc.gpsimd.memset(wsrc, 0.5)
        pwarm = warm.tile([128, 512], F32, tag="pwarm")
        for i in range(10):
            nc.tensor.matmul(pwarm, wsrc[:, 0:128], wsrc,
                             start=(i == 0), stop=(i == 9))

        # x^T rows: d = j*128 + p, free dims (j, n)
        xv = xt_d[:, :].rearrange("(j p) n -> p j n", p=128)

        x_const = None
        if skip_dma:
            x_const = constp.tile([128, 2 * BLK], FP16, tag="xconst")
            nc.gpsimd.memset(x_const, 0.25)

        str_const = None
        if skip_streams:
            str_const = constp.tile([128, 2 * BLK], FP16, tag="strconst")
            nc.gpsimd.memset(str_const, 0.125)

        def prepare_block(st, ntok):
            """DMA + tanh + all stream computation for tokens [st, st+ntok),
            in (d, n) layout throughout.  Tiles are allocated at the max
            block size; only the first 2*ntok columns are used, so the flat
            free layout is j*ntok + n."""
            if skip_streams:
                return [str_const] * NSTREAMS
            if skip_dma:
                x_in = x_const[:, 0:2 * ntok]
            else:
                xt_t = xin.tile([128, 2 * BLK], FP16, tag="x")
                nc.sync.dma_start(
                    out=xt_t[:, 0:2 * ntok].rearrange("p (j n) -> p j n", j=2),
                    in_=xv[:, :, st:st + ntok],
                )
                x_in = xt_t[:, 0:2 * ntok]
            w2 = 2 * ntok
            t = sp.tile([128, 2 * BLK], FP16, tag="t")
            nc.scalar.activation(out=t[:, 0:w2], in_=x_in, func=AF.Tanh)
            s = sp.tile([128, 2 * BLK], FP16, tag="s")
            nc.scalar.activation(out=s[:, 0:w2], in_=t[:, 0:w2], func=AF.Square)
            u = sp.tile([128, 2 * BLK], FP16, tag="u")
            nc.vector.tensor_scalar(out=u[:, 0:w2], in0=s[:, 0:w2],
                                    scalar1=2.0, scalar2=-1.0,
                                    op0=ALU.mult, op1=ALU.add)
            u2 = sp.tile([128, 2 * BLK], FP16, tag="u2")
            nc.scalar.activation(out=u2[:, 0:w2], in_=u[:, 0:w2], func=AF.Square)
            tu = sp.tile([128, 2 * BLK], FP16, tag="tu")
            nc.vector.tensor_tensor(out=tu[:, 0:w2], in0=t[:, 0:w2],
                                    in1=u[:, 0:w2], op=ALU.mult)
            u3 = sp.tile([128, 2 * BLK], FP16, tag="u3")
            nc.vector.tensor_tensor(out=u3[:, 0:w2], in0=u[:, 0:w2],
                                    in1=u2[:, 0:w2], op=ALU.mult)
            tu2 = sp.tile([128, 2 * BLK], FP16, tag="tu2")
            nc.vector.tensor_tensor(out=tu2[:, 0:w2], in0=t[:, 0:w2],
                                    in1=u2[:, 0:w2], op=ALU.mult)
            tu3 = sp.tile([128, 2 * BLK], FP16, tag="tu3")
            nc.vector.tensor_tensor(out=tu3[:, 0:w2], in0=tu[:, 0:w2],
                                    in1=u2[:, 0:w2], op=ALU.mult)
            return [t, u, tu, u2, tu2, u3, tu3]

        def mm_block(st, ntok, streams):
            if skip_mm:
                return
            nh_tot = (ntok + 511) // 512
            for og in range(2):
                pw = pyp.tile([128, BLK], F32, tag="pw")
                for nh in range(nh_tot):
                    n0, n1 = nh * 512, min((nh + 1) * 512, ntok)
                    kk = 0
                    for si in range(NSTREAMS):
                        for dc in range(2):
                            nc.tensor.matmul(
                                pw[:, n0:n1],
                                wt[si * 2 + dc][:, og * 128:(og + 1) * 128],
                                streams[si][:, dc * ntok + n0:dc * ntok + n1],
                                start=(kk == 0), stop=(kk == 2 * NSTREAMS - 1),
                            )
                            kk += 1
                yo = yp.tile([128, BLK], FP16, tag=f"yo{og}", name=f"yo{og}")
                # evac with per-partition f32 bias add; og0 on ACT, og1 on
                # DVE so neither FIFO stacks two PE-gated ops per block
                if og == 0:
                    nc.scalar.activation(out=yo[:, 0:ntok], in_=pw[:, 0:ntok],
                                         func=AF.Identity,
                                         bias=bias_og[og], scale=1.0)
                else:
                    nc.vector.tensor_scalar(out=yo[:, 0:ntok], in0=pw[:, 0:ntok],
                                            scalar1=bias_og[og], scalar2=None,
                                            op0=ALU.add)
                if not skip_dma:
                    nc.sync.dma_start(
                        out=yt[og * 128:(og + 1) * 128, st:st + ntok],
                        in_=yo[:, 0:ntok],
                    )

        # Tapered block list: small first/last blocks shorten pipeline fill
        # (first streams ready sooner) and drain (last evac+DMA smaller).
        if nblk >= 4:
            blocks = ([(0, 256), (256, 256)]
                      + [(512 + i * BLK, BLK) for i in range(nblk - 2)]
                      + [(nshard - 512, 256), (nshard - 256, 256)])
        else:
            blocks = [(i * BLK, BLK) for i in range(nblk)]
        assert sum(n for _, n in blocks) == nshard

        def run_pipeline():
            # lookahead blocks: stream chain latency (~6.5us incl DMA) is
            # about one MM-block (6us), so one block ahead is not enough.
            nb = len(blocks)
            pending = [prepare_block(*blocks[b]) for b in range(min(lookahead, nb))]
            for bI in range(nb):
                if bI + lookahead < nb:
                    pending.append(prepare_block(*blocks[bI + lookahead]))
                mm_block(*blocks[bI], pending.pop(0))

        if reps > 1:
            with tc.For_i(0, reps, 1):
                run_pipeline()
        else:
            run_pipeline()

    nc.compile()
    return nc


def kernel(x, cheby_coeffs, bias):
    global LAST_RESULTS
    # NTFF trace hooks (antenv.axon_hooks) are absent in this container;
    # make sure nothing flips tracing on under us.
    os.environ["BASS_NEVER_TRACE"] = "1"
    from concourse.bass_utils import run_bass_kernel_spmd

    x = np.asarray(x, dtype=np.float32)
    n_tok = x.shape[0]
    assert n_tok % NCORES == 0
    nshard = n_tok // NCORES

    W14, bt = _fold_weights(np.asarray(cheby_coeffs), np.asarray(bias))
    x16 = x.astype(np.float16)

    key = nshard
    if key not in _PROGRAM_CACHE:
        _PROGRAM_CACHE[key] = build_program(nshard)
    nc = _PROGRAM_CACHE[key]

    in_maps = [
        {"xt": np.ascontiguousarray(x16[c * nshard:(c + 1) * nshard].T),
         "w": W14, "bt": bt}
        for c in range(NCORES)
    ]
    res = run_bass_kernel_spmd(nc, in_maps, list(range(NCORES)))
    LAST_RESULTS = res
    y = np.concatenate(
        [res.results[c]["yt"].T.astype(np.float32) for c in range(NCORES)],
        axis=0,
    )
    return y
